# revision 6
# baseline (speedup 1.0000x reference)
"""Trainium2 Bass kernel for nn_APNLayer_38259568673551 (gated delta rule + linear + LN).

Self-contained: hardcodes shapes B=4, L=2048, D=512.

Algorithm (chunked parallel gated delta rule, chunk C=128):
  K = Q = tanh(x);  V = K @ W^T;  lam = sigmoid(lam_logit); beta = eta
  Per chunk c (state carried as Y = lam^C * S):
    G = K_c K_c^T
    T = inv(I + beta*stricttril(G)) via exact nilpotent product
        prod_k (I + X^(2^k)), X = -beta*stricttril(G)
    KS = K_c @ Y_{c-1}
    Delta = beta*T @ (TildeV - KS),  TildeV[j] = V[j] / lam^(j+1)
    y_c  = lam^(j+1) * (KS + tril_incl(G) @ Delta + TildeV)
    Y_c  = lam^C * (Y_{c-1} + K_c^T Delta)
  Then LayerNorm over D.

Sharding: 8 cores = 4 batches x 2 value-column halves (DV=256). State columns
evolve independently, so the scan is embarrassingly parallel across (b, half).
LayerNorm row stats are exchanged between the pair of cores of each batch with
a tiny AllReduce ([128,8] per slab of 4 row-tiles).
"""
import os
import numpy as np

B, L, D = 4, 2048, 512
C = 128            # chunk length
NCH = L // C       # 16 chunks
KT = D // 128      # 4 contraction tiles
DV = D // 2        # 256 value cols per core
NFAC = 5           # factors in the nilpotent product (exact to ~1e-6)
EPS = 1e-5
NSLAB = 4          # LN stat slabs (4 L-tiles each)

_CACHE = {}


def _build(lam, beta, affine, dt_main_name):
    import concourse.bacc as bacc
    import concourse.tile as tile
    from concourse import mybir

    f32 = mybir.dt.float32
    DT = getattr(mybir.dt, dt_main_name)
    ACTF = mybir.ActivationFunctionType
    ALU = mybir.AluOpType

    lamC = float(lam) ** C
    lamj = (np.float64(lam) ** np.arange(1, C + 1)).astype(np.float32)  # [C]

    nc = bacc.Bacc("TRN2", target_bir_lowering=False, debug=False, num_devices=8)

    x_nat = nc.dram_tensor("x_nat", [L, D], DT, kind="ExternalInput")
    x_tr = nc.dram_tensor("x_tr", [D, L], DT, kind="ExternalInput")
    whT = nc.dram_tensor("whT", [D, DV], DT, kind="ExternalInput")
    # constant matrices [128,128]: I, -I, masks (already scaled by -beta), triu-incl
    cmat = nc.dram_tensor("cmat", [5, 128, 128], DT, kind="ExternalInput")
    cvec = nc.dram_tensor("cvec", [128, 2], f32, kind="ExternalInput")  # lamj, -1/lamj
    if affine:
        gb = nc.dram_tensor("gb", [2, DV], f32, kind="ExternalInput")
    y_out = nc.dram_tensor("y_out", [L, DV], f32, kind="ExternalOutput")

    with tile.TileContext(nc) as tc:
        with (
            tc.tile_pool(name="const", bufs=1) as constp,
            tc.tile_pool(name="xa", bufs=1) as xap,
            tc.tile_pool(name="yp", bufs=1) as yp,
            tc.tile_pool(name="ld", bufs=3) as ldp,
            tc.tile_pool(name="tw", bufs=3) as twp,
            tc.tile_pool(name="sc", bufs=2) as scp,
            tc.tile_pool(name="zp", bufs=2) as zp,
            tc.tile_pool(name="lnp", bufs=4) as lnp,
            tc.tile_pool(name="ps_g", bufs=1, space="PSUM") as ps_g,
            tc.tile_pool(name="ps_t", bufs=2, space="PSUM") as ps_t,
            tc.tile_pool(name="ps_b", bufs=3, space="PSUM") as ps_b,
            tc.tile_pool(name="ps_z", bufs=1, space="PSUM") as ps_z,
            tc.tile_pool(name="dram", bufs=1, space="DRAM") as dramp,
        ):
            # ---- constants ----
            cm = constp.tile([128, 5, 128], DT, tag="cmat")
            nc.sync.dma_start(out=cm, in_=cmat.ap().rearrange("n p f -> p n f"))
            ident = cm[:, 0, :]
            nident = cm[:, 1, :]
            maskLo = cm[:, 2, :]   # -beta on strict lower
            maskUp = cm[:, 3, :]   # -beta on strict upper
            maskUI = cm[:, 4, :]   # 1.0 on upper incl diag
            epst = constp.tile([128, 1], f32, tag="eps")
            nc.vector.memset(epst, EPS)
            cv = constp.tile([128, 2], f32, tag="cvec")
            nc.sync.dma_start(out=cv, in_=cvec.ap())
            lamj_ap = cv[:, 0:1]
            nlamjinv_ap = cv[:, 1:2]
            wht_t = []
            for kt in range(KT):
                t = constp.tile([128, DV], DT, tag=f"wht{kt}")
                nc.sync.dma_start(out=t, in_=whT.ap()[128 * kt:128 * (kt + 1), :])
                wht_t.append(t)
            if affine:
                gmt = constp.tile([128, DV], f32, tag="gm")
                bmt = constp.tile([128, DV], f32, tag="bm")
                gbab = gb.ap()
                nc.sync.dma_start(out=gmt, in_=bass_bcast(gbab[0:1, :], 128))
                nc.sync.dma_start(out=bmt, in_=bass_bcast(gbab[1:2, :], 128))

            # ---- load + tanh both layouts ----
            # transposed layout: 16 tiles [128, 512] = xaT[kt][q]
            xaT = [[None] * 4 for _ in range(KT)]
            for kt in range(KT):
                for q in range(4):
                    raw = ldp.tile([128, 512], DT, tag="rawT")
                    nc.sync.dma_start(
                        out=raw,
                        in_=x_tr.ap()[128 * kt:128 * (kt + 1), 512 * q:512 * (q + 1)],
                    )
                    t = xap.tile([128, 512], DT, tag=f"xaT{kt}_{q}")
                    nc.scalar.activation(out=t, in_=raw, func=ACTF.Tanh)
                    xaT[kt][q] = t
            # natural layout: 16 tiles [128, 512] (tile c = rows of chunk c)
            xan = []
            for c in range(NCH):
                raw = ldp.tile([128, 512], DT, tag="rawN")
                nc.sync.dma_start(out=raw, in_=x_nat.ap()[128 * c:128 * (c + 1), :])
                t = xap.tile([128, 512], DT, tag=f"xan{c}")
                nc.scalar.activation(out=t, in_=raw, func=ACTF.Tanh)
                xan.append(t)

            def kslice(kt, c):
                # [128,128] lhsT slice of K_c^T in contraction tile kt
                return xaT[kt][c // 4][:, 128 * (c % 4):128 * (c % 4 + 1)]

            # ---- state init ----
            z_cur = []
            for mt in range(KT):
                zt = zp.tile([128, DV], DT, tag=f"z{mt}")
                nc.vector.memset(zt, 0.0)
                z_cur.append(zt)

            y_tiles = []
            stats_slab = None
            slab_tiles = []
            ncopy = [0]

            def psum_copy(dst, src, scale=None):
                """PSUM->SBUF copy, alternating DVE / ACT."""
                ncopy[0] += 1
                if ncopy[0] % 2 == 0:
                    if scale is None:
                        nc.vector.tensor_copy(dst, src)
                    else:
                        nc.vector.tensor_scalar_mul(dst, src, float(scale))
                else:
                    nc.scalar.activation(
                        out=dst, in_=src, func=ACTF.Copy,
                        scale=1.0 if scale is None else float(scale),
                    )

            for c in range(NCH):
                # ---- static V (negated TildeV) ----
                psV = ps_b.tile([128, DV], f32, tag="psb")
                for kt in range(KT):
                    nc.tensor.matmul(psV, kslice(kt, c), wht_t[kt],
                                     start=(kt == 0), stop=(kt == KT - 1))
                ntv = scp.tile([128, DV], DT, tag="ntv")
                # ntv = -TildeV = -V/lamj
                nc.vector.tensor_scalar_mul(ntv, psV, nlamjinv_ap)

                # ---- G ----
                psG = ps_g.tile([128, 128], f32, tag="psg")
                for kt in range(KT):
                    nc.tensor.matmul(psG, kslice(kt, c), kslice(kt, c),
                                     start=(kt == 0), stop=(kt == KT - 1))
                lo = twp.tile([128, 128], DT, tag="lo0")
                up = twp.tile([128, 128], DT, tag="up0")
                tri = twp.tile([128, 128], DT, tag="tri")
                nc.vector.tensor_tensor(lo, psG, maskLo, op=ALU.mult)
                nc.vector.tensor_tensor(up, psG, maskUp, op=ALU.mult)
                nc.vector.tensor_tensor(tri, psG, maskUI, op=ALU.mult)

                # ---- T^T via nilpotent product ----
                # P = I + U; P += U^(2^k) @ P  (lhsT = lower-power = lo_k)
                psP = ps_t.tile([128, 128], f32, tag="pst")
                nc.tensor.matmul(psP, ident, ident, start=True, stop=False)
                nc.tensor.matmul(psP, lo, ident, start=False, stop=True)
                pp = twp.tile([128, 128], DT, tag="pp")
                psum_copy(pp, psP)
                for k in range(1, NFAC):
                    last = k == NFAC - 1
                    # square: lo2 = up@lo (lhsT=up... lhsT.T@rhs = lo.T.T?? ):
                    # lo_{k} = X^(2^k) lower; up_{k} upper. lo_k = mm(lhsT=up_{k-1}, rhs=lo_{k-1})
                    psq = ps_t.tile([128, 128], f32, tag="pst")
                    nc.tensor.matmul(psq, up, lo, start=True, stop=True)
                    lo2 = twp.tile([128, 128], DT, tag="lo")
                    psum_copy(lo2, psq)
                    if not last:
                        psq2 = ps_t.tile([128, 128], f32, tag="pst")
                        nc.tensor.matmul(psq2, lo, up, start=True, stop=True)
                        up2 = twp.tile([128, 128], DT, tag="up")
                        psum_copy(up2, psq2)
                        up = up2
                    lo = lo2
                    # product update: P = P + U^(2^k) @ P
                    psP = ps_t.tile([128, 128], f32, tag="pst")
                    nc.tensor.matmul(psP, ident, pp, start=True, stop=False)
                    nc.tensor.matmul(psP, lo, pp, start=False, stop=True)
                    pp = twp.tile([128, 128], DT, tag="pp")
                    # last: scale by -beta -> nbTT = -beta * T^T
                    psum_copy(pp, psP, scale=(-float(beta) if last else None))
                nbTT = pp

                # ---- KS ----
                psKS = ps_b.tile([128, DV], f32, tag="psb")
                for kt in range(KT):
                    nc.tensor.matmul(psKS, kslice(kt, c), z_cur[kt],
                                     start=(kt == 0), stop=(kt == KT - 1))
                ks_s = scp.tile([128, DV], DT, tag="ks")
                psum_copy(ks_s, psKS)

                # ---- Delta = beta*T@(TildeV - KS) = (-beta T)( -TildeV) + (-beta T)(KS) ----
                psD = ps_b.tile([128, DV], f32, tag="psb")
                nc.tensor.matmul(psD, nbTT, ntv, start=True, stop=False)
                nc.tensor.matmul(psD, nbTT, ks_s, start=False, stop=True)
                dl_s = scp.tile([128, DV], DT, tag="dl")
                psum_copy(dl_s, psD)

                # ---- O accumulation onto psKS, then y ----
                nc.tensor.matmul(psKS, tri, dl_s, start=False, stop=False,
                                 skip_group_check=True)
                nc.tensor.matmul(psKS, nident, ntv, start=False, stop=True,
                                 skip_group_check=True)
                yt = yp.tile([128, DV], f32, tag=f"y{c}")
                nc.scalar.activation(out=yt, in_=psKS, func=ACTF.Copy, scale=lamj_ap)
                y_tiles.append(yt)

                # ---- LN stats ----
                if c % 4 == 0:
                    stats_slab = lnp.tile([128, 8], f32, tag="slab")
                st6 = lnp.tile([128, 6], f32, tag="st6")
                nc.vector.bn_stats(out=st6, in_=yt)
                nc.vector.bn_aggr(out=stats_slab[:, 2 * (c % 4):2 * (c % 4) + 2], in_=st6)
                if c % 4 == 3:
                    slab_tiles.append(stats_slab)

                # ---- state update: Y_new = lamC*(Y + K^T Delta) ----
                psZ = ps_z.tile([128, KT, DV], f32, tag="psz")
                for mt in range(KT):
                    nc.tensor.matmul(psZ[:, mt, :], xan[c][:, 128 * mt:128 * (mt + 1)],
                                     dl_s, start=True, stop=False)
                    nc.tensor.matmul(psZ[:, mt, :], ident, z_cur[mt],
                                     start=False, stop=True)
                z_new = []
                for mt in range(KT):
                    zt = zp.tile([128, DV], DT, tag=f"z{mt}")
                    psum_copy(zt, psZ[:, mt, :], scale=lamC)
                    z_new.append(zt)
                z_cur = z_new

            # ---- LN finalize per slab ----
            for s in range(NSLAB):
                slab = slab_tiles[s]
                sl = slab.rearrange("p (a two) -> p a two", two=2)
                means, var_ = sl[:, :, 0], sl[:, :, 1]
                pay = lnp.tile([128, 8], f32, tag="pay")
                nc.vector.tensor_tensor(pay[:, 4:8], means, means, op=ALU.mult)
                nc.vector.tensor_tensor(pay[:, 4:8], pay[:, 4:8], var_, op=ALU.add)
                nc.vector.tensor_copy(pay[:, 0:4], means)
                din = dramp.tile([128, 8], f32, tag=f"din{s}")
                dout = dramp.tile([128, 8], f32, tag=f"dout{s}")
                nc.sync.dma_start(out=din, in_=pay)
                nc.gpsimd.collective_compute(
                    "AllReduce", ALU.add,
                    replica_groups=[[0, 1], [2, 3], [4, 5], [6, 7]],
                    ins=[din.opt()], outs=[dout.opt()],
                )
                red = lnp.tile([128, 8], f32, tag="red")
                nc.sync.dma_start(out=red, in_=dout)
                mu4 = lnp.tile([128, 4], f32, tag="mu4")
                nc.vector.tensor_scalar_mul(mu4, red[:, 0:4], 0.5)
                var4 = lnp.tile([128, 4], f32, tag="var4")
                # var = 0.5*sum_m2 - mu^2
                nc.vector.tensor_scalar_mul(var4, red[:, 4:8], 0.5)
                musq = lnp.tile([128, 4], f32, tag="musq")
                nc.vector.tensor_tensor(musq, mu4, mu4, op=ALU.mult)
                nc.vector.tensor_tensor(var4, var4, musq, op=ALU.subtract)
                sd = lnp.tile([128, 4], f32, tag="sd")
                nc.scalar.activation(out=sd, in_=var4, func=ACTF.Sqrt, bias=epst)
                rstd = lnp.tile([128, 4], f32, tag="rstd")
                nc.vector.reciprocal(rstd, sd)
                for i in range(4):
                    c = 4 * s + i
                    yt = y_tiles[c]
                    nc.vector.tensor_scalar(
                        out=yt, in0=yt,
                        scalar1=mu4[:, i:i + 1], scalar2=rstd[:, i:i + 1],
                        op0=ALU.subtract, op1=ALU.mult,
                    )
                    if affine:
                        nc.vector.tensor_tensor(yt, yt, gmt, op=ALU.mult)
                        nc.vector.tensor_tensor(yt, yt, bmt, op=ALU.add)
                    nc.sync.dma_start(out=y_out.ap()[128 * c:128 * (c + 1), :], in_=yt)

    nc.finalize()
    return nc


def bass_bcast(ap, p):
    """Broadcast a [1, N] AP along partitions."""
    import concourse.bass as bass
    return bass.AP(tensor=ap.tensor, offset=ap.offset, ap=[[0, p]] + list(ap.ap[1:]))


def _np_dt(name):
    import ml_dtypes
    return np.float32 if name == "float32" else ml_dtypes.bfloat16


def kernel(x, W, eta, lam_logit, ln_gamma, ln_beta):
    from concourse.bass_utils import run_bass_kernel_spmd

    dt_name = os.environ.get("APN_DT", "float32")
    ndt = _np_dt(dt_name)

    x = np.asarray(x, np.float32)
    W = np.asarray(W, np.float32)
    lam = float(1.0 / (1.0 + np.exp(-np.float64(lam_logit))))
    beta = float(np.float64(eta))
    g = np.asarray(ln_gamma, np.float32)
    bb = np.asarray(ln_beta, np.float32)
    affine = not (np.allclose(g, 1.0) and np.allclose(bb, 0.0))

    key = (dt_name, affine, round(lam, 12), round(beta, 12))
    if key not in _CACHE:
        _CACHE[key] = _build(lam, beta, affine, dt_name)
    nc = _CACHE[key]

    # constants
    lamj = (np.float64(lam) ** np.arange(1, C + 1)).astype(np.float32)
    cvec = np.stack([lamj, -1.0 / lamj], axis=1).astype(np.float32)  # [128,2]
    I = np.eye(128, dtype=np.float32)
    sl = np.tril(np.ones((128, 128), np.float32), -1)
    cmat = np.stack([
        I, -I, -beta * sl, -beta * sl.T, np.triu(np.ones((128, 128), np.float32), 0),
    ]).astype(ndt)

    in_maps = []
    for core in range(8):
        b, h = core // 2, core % 2
        xb = np.ascontiguousarray(x[b]).astype(ndt)
        xbT = np.ascontiguousarray(x[b].T).astype(ndt)
        whT = np.ascontiguousarray(W[h * DV:(h + 1) * DV, :].T).astype(ndt)
        m = {"x_nat": xb, "x_tr": xbT, "whT": whT, "cmat": cmat, "cvec": cvec}
        if affine:
            m["gb"] = np.stack([g[h * DV:(h + 1) * DV], bb[h * DV:(h + 1) * DV]])
        in_maps.append(m)

    res = run_bass_kernel_spmd(nc, in_maps, core_ids=list(range(8)))
    out = np.empty((B, L, D), np.float32)
    for core in range(8):
        b, h = core // 2, core % 2
        out[b, :, h * DV:(h + 1) * DV] = res.results[core]["y_out"]
    return out


if __name__ == "__main__":
    d = np.load("/root/problem/inputs_cache.npz")
    inputs = {k: d[k] for k in d.files}
    got = kernel(**inputs)
    exp = np.load("/root/problem/expected_cache.npy")
    am = np.abs(got - exp).max() / np.abs(exp).max()
    rms = float(np.sqrt(((got - exp) ** 2).sum() / (exp ** 2).sum()))
    print(f"KERNEL absmax-rel={am:.3e} rms-rel={rms:.3e}")
